# revision 29
# baseline (speedup 1.0000x reference)
"""Trainium2 Bass kernel: full-embed-dim self-attention + residual LayerNorm.

Problem: B=4, S=2048, D=1024 fp32.
  q/k/v = x@w{q,k,v}+b; scores = q@k^T/sqrt(D); attn = softmax(scores)@v;
  out = LN(x + attn@wo + bo) * gamma + beta.

Sharding: 8 cores = 4 batches x 2 query-halves (1024 queries each), no
collectives: each core receives the full batch sequence x plus the
associativity-folded weight products
  wqk = wq @ wk^T   (scores = x wqk x^T)
  wvo = wv @ wo     (attn out = (P @ x) @ wvo / denom)
eliminating the K/V projections (6.44e9 vs 8.59e9 MACs per core). Keys
are shipped permuted (own query half first) so offsets are static;
softmax reductions are key-order invariant.

All four big matmul groups run in fp8(e4m3) DoubleRow mode: operands are
pair-packed [128, 2, N] so each PE instruction contracts 256 rows
(~1.44x bf16 throughput at FD=512). Host scales (8x wqk, 16x wvo) keep
fp8 operands in the normal range; exp() absorbs 1/(32*8) and a -3 bias
so PT fits e4m3's 240 max; the 16x on wvo cancels via ones=16 in the
denominator matmul. fp32 PSUM throughout; residual + LayerNorm in fp32
with only the final store rounded to bf16 (host upcasts).

Per-core dataflow (d-on-partitions):
  TT[do,q]  = wqk^T @ xq^T                      (A: 64 DR-matmuls)
  ST[k,q]   = x^T^T @ TT ; PT = exp(ST/256+cb)  (B: 128)
  den[q]    = 16*ones^T @ PT  -> DRAM-transposed -> recip[q-part]
  YT[d,q]   = x^T @ PT                          (C: 128)
  O[q,e]    = YT^T @ wvo                        (E: 64), then
  v = O*recip + xq_aug; LayerNorm (fused tensor_scalar when gamma==1,
  beta==0, else general two-op path).
"""

import numpy as np
import ml_dtypes

import concourse.bass as bass
import concourse.mybir as mybir
import concourse.tile as tile
from concourse import bacc

F32 = mybir.dt.float32
BF16 = mybir.dt.bfloat16
FP8 = mybir.dt.float8e4
DR = mybir.MatmulPerfMode.DoubleRow
NPF8 = ml_dtypes.float8_e4m3

B, S, D = 4, 2048, 1024
Q = 1024            # queries per core
WQK_S = 8.0         # host scale on wqk (fp8 range)
WVO_S = 16.0        # host scale on wvo; cancelled by ones=16 in denom
SCALE = 1.0 / (32.0 * WQK_S)   # exp scale: 1/sqrt(D) / WQK_S
CSHIFT = 3.0        # logit shift so PT max ~e^2.5 << 240 (e4m3 max)
EPS = 1e-6
NKC = S // 128      # 16 key chunks
NDC = D // 128      # 8 d chunks
NKP = NKC // 2      # 8 key pairs
NDP = NDC // 2      # 4 d pairs


def _bcast_ap(ap_1d, parts=128):
    """[N] dram AP -> [parts, N] AP with 0-stride partition dim."""
    return bass.AP(
        tensor=ap_1d.tensor, offset=ap_1d.offset, ap=[[0, parts]] + list(ap_1d.ap)
    )


def _transpose_ap(ap_1d, parts=128, cols=8):
    """[parts*cols] dram AP -> [parts, cols] AP: out[p, c] = in[c*parts + p]."""
    return bass.AP(
        tensor=ap_1d.tensor, offset=ap_1d.offset, ap=[[1, parts], [parts, cols]]
    )


def build_nc(fused_ln: bool):
    nc = bacc.Bacc("TRN2", target_bir_lowering=False, debug=False, num_devices=8)

    xtpo_d = nc.dram_tensor("xtpo", [4 * 128, 2, Q], FP8, kind="ExternalInput")
    xtpp_d = nc.dram_tensor("xtpp", [4 * 128, 2, Q], FP8, kind="ExternalInput")
    xkp_d = nc.dram_tensor("xkp", [8 * 128, 2, D], FP8, kind="ExternalInput")
    wqkp_d = nc.dram_tensor("wqkp", [4 * 128, 2, D], FP8, kind="ExternalInput")
    wvop_d = nc.dram_tensor("wvop", [4 * 128, 2, D], FP8, kind="ExternalInput")
    xq_d = nc.dram_tensor("xq", [Q, D], F32, kind="ExternalInput")
    cb_d = nc.dram_tensor("cb", [128, NKC], F32, kind="ExternalInput")
    gamma_d = nc.dram_tensor("gamma", [D], BF16, kind="ExternalInput")
    beta_d = nc.dram_tensor("beta", [D], BF16, kind="ExternalInput")
    out_d = nc.dram_tensor("out", [Q, D], BF16, kind="ExternalOutput")

    with tile.TileContext(nc) as tc:
        with (
            tc.tile_pool(name="small", bufs=1) as p_small,
            tc.tile_pool(name="dram", bufs=1, space="DRAM") as p_dram,
            tc.tile_pool(name="ptp", bufs=NKP) as p_pt,
            tc.tile_pool(name="xkp", bufs=NKP) as p_xk,
            tc.tile_pool(name="ytp", bufs=NDP) as p_yt,
        ):
            den_dram = p_dram.tile([Q], F32, name="den_dram")

            cb = p_small.tile([128, NKC], F32)
            nc.gpsimd.dma_start(out=cb[:, :], in_=cb_d[:, :])
            ones16 = p_small.tile([128, 2, 128], FP8)
            nc.vector.memset(ones16[:, :, :], WVO_S)
            warm = p_small.tile([128, 512], BF16)
            nc.vector.memset(warm[:, :], 0.0)
            eps_t = p_small.tile([128, 1], F32)
            nc.vector.memset(eps_t[:, :], EPS)
            den_sb = p_small.tile([1, Q], F32)
            recip = p_small.tile([128, 8], F32)

            ptp = [p_pt.tile([128, 2, Q], FP8, tag="ptp", name=f"ptp{i}") for i in range(NKP)]
            ytp = [p_yt.tile([128, 2, Q], FP8, tag="ytp", name=f"ytp{i}") for i in range(NDP)]
            xkp = [p_xk.tile([128, 2, D], FP8, tag="xkp", name=f"xkp{i}") for i in range(NKP)]

            # ---- stages A+B(+D): TT, then ST->exp->PT, then denominators ----
            with (
                tc.tile_pool(name="xtp", bufs=2 * NDP) as p_xt,
                tc.tile_pool(name="ttp", bufs=NDP) as p_tt,
                tc.tile_pool(name="psA", bufs=6, space="PSUM") as p_psA,
                tc.tile_pool(name="psD", bufs=2, space="PSUM") as p_psD,
            ):
                # x^T pairs, split own-query / partner halves (flat 2KB DMAs)
                xto = [p_xt.tile([128, 2, Q], FP8, tag="xt", name=f"xto{i}") for i in range(NDP)]
                xtp = [p_xt.tile([128, 2, Q], FP8, tag="xt", name=f"xtq{i}") for i in range(NDP)]
                ttp = [p_tt.tile([128, 2, Q], FP8, tag="ttp", name=f"ttp{i}") for i in range(NDP)]

                # HAM pre-warm: dummy matmuls during the input-DMA dead time so
                # the PE clock gate is already 8/8 when the real stream starts.
                ps_w = p_psD.tile([128, 512], F32, tag="psD", name="ps_warm")
                for i in range(9):
                    nc.tensor.matmul(
                        ps_w[:, :], warm[:, 0:128], warm[:, :],
                        start=(i == 0), stop=(i == 8),
                    )
                with tc.tile_pool(name="wqkp", bufs=NDP) as p_wqk:
                    wqkp = [p_wqk.tile([128, 2, D], FP8, tag="wqkp", name=f"wqkp{i}") for i in range(NDP)]
                    for j in range(NDP):
                        nc.sync.dma_start(out=xto[j][:, :, :], in_=xtpo_d[128 * j:128 * (j + 1), :, :])
                        nc.gpsimd.dma_start(out=wqkp[j][:, :, :], in_=wqkp_d[128 * j:128 * (j + 1), :, :])
                    for j in range(NDP):
                        nc.scalar.dma_start(out=xtp[j][:, :, :], in_=xtpp_d[128 * j:128 * (j + 1), :, :])

                    # TT: j-outermost over PSUM banks so the PE starts once
                    # the first (wqkp, xto) pair lands.
                    for qh in range(2):
                        pss = [p_psA.tile([128, 512], F32, tag="psA", name=f"pstt{qh}_{do}") for do in range(6)]
                        pss += [p_psD.tile([128, 512], F32, tag="psD", name=f"psttd{qh}_{do}") for do in range(2)]
                        for j in range(NDP):
                            for do in range(NDC):
                                nc.tensor.matmul(
                                    pss[do][:, :],
                                    wqkp[j][:, :, 128 * do:128 * (do + 1)],
                                    xto[j][:, :, 512 * qh:512 * (qh + 1)],
                                    start=(j == 0), stop=(j == NDP - 1),
                                    perf_mode=DR,
                                )
                        for do in range(NDC):
                            nc.vector.tensor_copy(ttp[do // 2][:, do % 2, 512 * qh:512 * (qh + 1)], pss[do][:, :])

                # prefetch x (key-major) for stage C while B runs
                for j in range(NKP):
                    nc.sync.dma_start(out=xkp[j][:, :, :], in_=xkp_d[128 * j:128 * (j + 1), :, :])

                # ST -> exp -> PT
                for qh in range(2):
                    for kc in range(NKC):
                        ps = p_psA.tile([128, 512], F32, tag="psA")
                        for j in range(NDP):
                            src = xto[j] if kc < NDC else xtp[j]
                            kl = kc % NDC
                            nc.tensor.matmul(
                                ps[:, :],
                                src[:, :, 128 * kl:128 * (kl + 1)],
                                ttp[j][:, :, 512 * qh:512 * (qh + 1)],
                                start=(j == 0), stop=(j == NDP - 1),
                                perf_mode=DR,
                            )
                        nc.scalar.activation(
                            out=ptp[kc // 2][:, kc % 2, 512 * qh:512 * (qh + 1)], in_=ps[:, :],
                            func=mybir.ActivationFunctionType.Exp,
                            bias=cb[:, kc:kc + 1], scale=SCALE,
                        )

                # denominators: ones(=16) lhsT duplicates den[q] on all
                # partitions; bounce via DRAM to transpose to [q-part, 8].
                # On the 2-bank psD pool so stage C's bank handoff (psA -> psC)
                # is not gated by the denominator drain.
                for qh in range(2):
                    psd = p_psD.tile([128, 512], F32, tag="psD")
                    for j in range(NKP):
                        nc.tensor.matmul(
                            psd[:, :],
                            ones16[:, :, :],
                            ptp[j][:, :, 512 * qh:512 * (qh + 1)],
                            start=(j == 0), stop=(j == NKP - 1),
                            perf_mode=DR,
                        )
                    nc.vector.tensor_copy(den_sb[0:1, 512 * qh:512 * (qh + 1)], psd[0:1, :])
                nc.sync.dma_start(out=den_dram[:], in_=den_sb[0:1, :])
                nc.sync.dma_start(out=recip[:, :], in_=_transpose_ap(den_dram[:]))
                nc.vector.reciprocal(recip[:, :], recip[:, :])

            # ---- stage C: YT[d, q] = x^T @ PT ----
            with (
                tc.tile_pool(name="wvop", bufs=NDP) as p_wvo,
                tc.tile_pool(name="xqp", bufs=8) as p_xq,
                tc.tile_pool(name="vout", bufs=6) as p_vo,
                tc.tile_pool(name="lnst", bufs=4) as p_ln,
            ):
                wvop = [p_wvo.tile([128, 2, D], FP8, tag="wvop", name=f"wvop{i}") for i in range(NDP)]
                for j in range(NDP):
                    nc.gpsimd.dma_start(out=wvop[j][:, :, :], in_=wvop_d[128 * j:128 * (j + 1), :, :])
                # prefetch all residual slabs during stage C (the gpsimd DMA
                # queue otherwise drains mid-stage-E)
                xqs = []
                for qp in range(8):
                    t = p_xq.tile([128, D], F32, tag="xq", name=f"xq{qp}")
                    xqs.append(t)
                    nc.gpsimd.dma_start(out=t[:, :], in_=xq_d[128 * qp:128 * (qp + 1), :])
                gam = p_small.tile([128, D], BF16)
                nc.gpsimd.dma_start(out=gam[:, :], in_=_bcast_ap(gamma_d[:]))
                bet = p_small.tile([128, D], BF16)
                nc.gpsimd.dma_start(out=bet[:, :], in_=_bcast_ap(beta_d[:]))

                with tc.tile_pool(name="psC", bufs=8, space="PSUM") as p_psC:
                    for qh in range(2):
                        for dc in range(NDC):
                            ps = p_psC.tile([128, 512], F32, tag="psC")
                            for j in range(NKP):
                                nc.tensor.matmul(
                                    ps[:, :],
                                    xkp[j][:, :, 128 * dc:128 * (dc + 1)],
                                    ptp[j][:, :, 512 * qh:512 * (qh + 1)],
                                    start=(j == 0), stop=(j == NKP - 1),
                                    perf_mode=DR,
                                )
                            nc.vector.tensor_copy(ytp[dc // 2][:, dc % 2, 512 * qh:512 * (qh + 1)], ps[:, :])

                # ---- stage E: O = YT^T @ wvo; v = O*recip + xq; LayerNorm ----
                # PSUM tiles span 2 banks so one fused DVE op covers both
                # matmul halves; small stats are fused tensor_scalars on DVE;
                # the final normalize alternates DVE / ACT(Identity) per qp.
                # Small LN stats are batched per qp-PAIR ([128,2] DVE ops) in a
                # shared tile: c0,1=sum(v); c2,3=sum(v^2); c4,5=-mean;
                # c6,7=mean^2; c8,9=var->rstd; c10,11=-mean*rstd.
                with tc.tile_pool(name="ps", bufs=4, space="PSUM") as p_ps:
                    vs = [None] * 8
                    sts = [None] * 4
                    for qp in range(8):
                        p2, h = qp // 2, qp % 2
                        v = p_vo.tile([128, D], F32, tag="v")
                        vs[qp] = v
                        sqs = p_vo.tile([128, D], F32, tag="sqs")
                        if h == 0:
                            sts[p2] = p_ln.tile([128, 12], F32, tag="st", name=f"st{p2}")
                        st = sts[p2]
                        ps = p_ps.tile([128, D], F32, tag="ps")
                        for eh in range(2):
                            for j in range(NDP):
                                nc.tensor.matmul(
                                    ps[:, 512 * eh:512 * (eh + 1)],
                                    ytp[j][:, :, 128 * qp:128 * (qp + 1)],
                                    wvop[j][:, :, 512 * eh:512 * (eh + 1)],
                                    start=(j == 0), stop=(j == NDP - 1),
                                    perf_mode=DR,
                                )
                        # v = O/(16*denom) + xq_aug; accum = sum(v)
                        nc.vector.scalar_tensor_tensor(
                            out=v[:, :], in0=ps[:, :],
                            scalar=recip[:, qp:qp + 1],
                            in1=xqs[qp][:, :],
                            op0=mybir.AluOpType.mult, op1=mybir.AluOpType.add,
                            accum_out=st[:, h:h + 1],
                        )
                        # E[v^2] via ACT Square + free accum = sum(v^2)
                        nc.scalar.activation(
                            out=sqs[:, :], in_=v[:, :],
                            func=mybir.ActivationFunctionType.Square,
                            accum_out=st[:, 2 + h:3 + h],
                        )
                        if h == 0:
                            continue
                        # both accums of the pair are in: batched stats
                        nc.vector.tensor_scalar_mul(st[:, 4:6], st[:, 0:2], -1.0 / D)
                        nc.vector.tensor_mul(st[:, 6:8], st[:, 4:6], st[:, 4:6])
                        nc.vector.tensor_scalar(
                            out=st[:, 8:10], in0=st[:, 2:4],
                            scalar1=1.0 / D, scalar2=None,
                            op0=mybir.AluOpType.mult,
                        )
                        nc.vector.tensor_sub(st[:, 8:10], st[:, 8:10], st[:, 6:8])
                        nc.scalar.activation(
                            out=st[:, 8:10], in_=st[:, 8:10],
                            func=mybir.ActivationFunctionType.Sqrt,
                            bias=eps_t[:, :],
                        )
                        nc.vector.reciprocal(st[:, 8:10], st[:, 8:10])     # rstd
                        nc.vector.tensor_mul(st[:, 10:12], st[:, 4:6], st[:, 8:10])
                        for qq in (qp - 1, qp):
                            hq = qq % 2
                            vout = p_vo.tile([128, D], BF16, tag="vout")
                            if fused_ln:
                                # gamma==1, beta==0: out = (v - mean) * rstd;
                                # alternate DVE / ACT(Identity) to balance
                                if hq == 0:
                                    nc.vector.tensor_scalar(
                                        out=vout[:, :], in0=vs[qq][:, :],
                                        scalar1=st[:, 4 + hq:5 + hq],
                                        scalar2=st[:, 8 + hq:9 + hq],
                                        op0=mybir.AluOpType.add,
                                        op1=mybir.AluOpType.mult,
                                    )
                                else:
                                    nc.scalar.activation(
                                        out=vout[:, :], in_=vs[qq][:, :],
                                        func=mybir.ActivationFunctionType.Identity,
                                        scale=st[:, 8 + hq:9 + hq],
                                        bias=st[:, 10 + hq:11 + hq],
                                    )
                            else:
                                # out = ((v + negmean)*gamma)*rstd + beta
                                nc.vector.scalar_tensor_tensor(
                                    out=vs[qq][:, :], in0=vs[qq][:, :],
                                    scalar=st[:, 4 + hq:5 + hq],
                                    in1=gam[:, :],
                                    op0=mybir.AluOpType.add, op1=mybir.AluOpType.mult,
                                )
                                nc.vector.scalar_tensor_tensor(
                                    out=vout[:, :], in0=vs[qq][:, :],
                                    scalar=st[:, 8 + hq:9 + hq],
                                    in1=bet[:, :],
                                    op0=mybir.AluOpType.mult, op1=mybir.AluOpType.add,
                                )
                            nc.sync.dma_start(out=out_d[128 * qq:128 * (qq + 1), :], in_=vout[:, :])
    nc.compile()
    return nc


_NC_CACHE = {}


def _pair_pack(a, np_chunks):
    """[np_chunks*256, N] -> [np_chunks*128, 2, N] DoubleRow pair layout."""
    n = a.shape[1]
    return np.ascontiguousarray(
        a.reshape(np_chunks, 2, 128, n).transpose(0, 2, 1, 3).reshape(np_chunks * 128, 2, n)
    )


def make_in_maps(inputs):
    x = np.asarray(inputs["inputs"], np.float32)
    wq = np.asarray(inputs["wq"], np.float32)
    wk = np.asarray(inputs["wk"], np.float32)
    wv = np.asarray(inputs["wv"], np.float32)
    wo = np.asarray(inputs["wo"], np.float32)
    bq = np.asarray(inputs["bq"], np.float32)
    f8 = lambda a: np.clip(a, -240.0, 240.0).astype(NPF8)
    bf = lambda a: np.ascontiguousarray(a).astype(ml_dtypes.bfloat16)
    bo_eff = np.asarray(inputs["bo"], np.float32) + np.asarray(inputs["bv"], np.float32) @ wo
    wqk = (wq @ wk.T) * WQK_S
    wvo = (wv @ wo) * WVO_S
    kcv = wk @ bq  # per-key logit offset direction: c[k] = x_k . kcv / 32
    shared = {
        "wqkp": f8(_pair_pack(wqk, NDP)),
        "wvop": f8(_pair_pack(wvo, NDP)),
        "gamma": bf(inputs["gamma"]),
        "beta": bf(inputs["beta"]),
    }
    in_maps = []
    for c in range(8):
        b, qh = c // 2, c % 2
        xb = x[b]                                  # [S, D]
        own = xb[Q * qh:Q * (qh + 1), :]
        oth = xb[Q * (1 - qh):Q * (2 - qh), :]
        xbp = np.concatenate([own, oth], axis=0)   # keys permuted: own half first
        cb = ((xbp @ kcv) / 32.0 - CSHIFT).reshape(NKC, 128).T
        xt_pairs = f8(_pair_pack(xbp.T, NDP))       # [512, 2, 2048]
        in_maps.append({
            **shared,
            "xtpo": np.ascontiguousarray(xt_pairs[:, :, 0:Q]),
            "xtpp": np.ascontiguousarray(xt_pairs[:, :, Q:S]),
            "xkp": f8(_pair_pack(xbp, NKP)),
            "xq": np.ascontiguousarray(own) + bo_eff[None, :],
            "cb": np.ascontiguousarray(cb),
        })
    return in_maps


def kernel(**inputs) -> np.ndarray:
    from concourse.bass_utils import run_bass_kernel_spmd

    fused_ln = bool(
        np.all(np.asarray(inputs["gamma"]) == 1.0)
        and np.all(np.asarray(inputs["beta"]) == 0.0)
    )
    if fused_ln not in _NC_CACHE:
        _NC_CACHE[fused_ln] = build_nc(fused_ln)
    res = run_bass_kernel_spmd(
        _NC_CACHE[fused_ln], make_in_maps(inputs), core_ids=list(range(8))
    )
    out = np.empty((B, S, D), np.float32)
    for c in range(8):
        b, qh = c // 2, c % 2
        out[b, Q * qh:Q * (qh + 1), :] = res.results[c]["out"].astype(np.float32)
    return out
